# revision 13
# baseline (speedup 1.0000x reference)
"""Trainium2 Bass kernel for per-sample Brownian-distance-covariance (BDC) pooling.

Problem: x [128, 640, 100] f32, t [1,1] f32 (log temperature).
  per sample: G = x @ x^T; dcov = d_i + d_j - 2G; dcov = max(dcov, 1e-4);
  z = sqrt(exp(t)*dcov + 1e-5); out = z - rowmean - colmean + totmean.
Output: [128, 409600] f32.

Strategy (8 NeuronCores, pure data parallel, 16 samples/core):
  - x cast to bf16; Gram via TensorE (contraction over the 100-dim on
    partitions, operands transposed on-chip via TensorE identity-matmul).
  - The per-sample distance matrix is symmetric -> column means == row means,
    so no column reduction is needed: row sums come free from the ScalarE
    sqrt pass (activation accum_out).
  - d_j enters the Gram PSUM via a rank-1 bf16 matmul of the transposed
    d-vector; the per-partition activation bias compensates the bf16
    rounding of that row exactly on the diagonal, so the clamp is never
    active off-diagonal and the diagonal lands at sqrt(1e-5) (vs reference
    sqrt(exp(t)*1e-4 + 1e-5) -- a 7.8e-5 absolute difference on 640 of
    409600 elements, far below tolerance).
  - Double centering fused into one VectorE scalar_tensor_tensor pass:
    out = (z - (rowmean[p] - totmean)) - M[p, e], with M = colmean broadcast
    built by tiny rank-2 matmuls from a bf16 hi/lo split of the row means.
"""
import numpy as np
from contextlib import ExitStack

import concourse.bass as bass
import concourse.bacc as bacc
import concourse.tile as tile
from concourse import mybir
from concourse.bass_utils import run_bass_kernel_spmd

F32 = mybir.dt.float32
BF16 = mybir.dt.bfloat16
AF = mybir.ActivationFunctionType
OP = mybir.AluOpType

N_CORES = 8
B_TOTAL = 128
B_CORE = B_TOTAL // N_CORES  # 16
DIM = 640
M = 100
NCHUNK = DIM // 128  # 5
GRP = 4              # samples per phase-group
NGRP = B_CORE // GRP

_cached_nc = None


def build():
    nc = bacc.Bacc("TRN2", target_bir_lowering=False)
    x = nc.dram_tensor("x", [B_CORE, DIM, M], F32, kind="ExternalInput")
    consts = nc.dram_tensor("consts", [128, 2], F32, kind="ExternalInput")
    ident_in = nc.dram_tensor("ident", [128, 128], F32, kind="ExternalInput")
    out = nc.dram_tensor("out", [B_CORE, DIM * DIM], F32, kind="ExternalOutput")

    with tile.TileContext(nc) as tc, ExitStack() as ctx:
        const_p = ctx.enter_context(tc.tile_pool(name="const", bufs=1))
        sb = ctx.enter_context(tc.tile_pool(name="sb", bufs=2))
        zpool = ctx.enter_context(tc.tile_pool(name="zp", bufs=6))
        xtp = ctx.enter_context(tc.tile_pool(name="xtp", bufs=6))
        psamp = ctx.enter_context(tc.tile_pool(name="psamp", bufs=8))
        opool = ctx.enter_context(tc.tile_pool(name="op", bufs=4))
        gp = ctx.enter_context(tc.tile_pool(name="gp", bufs=2))
        ps_gram = ctx.enter_context(tc.tile_pool(name="psg", bufs=2, space="PSUM"))
        ps_m = ctx.enter_context(tc.tile_pool(name="psm", bufs=1, space="PSUM"))
        ps_xp = ctx.enter_context(tc.tile_pool(name="psx", bufs=1, space="PSUM"))

        # ---- constants ----
        c_consts = const_p.tile([128, 2], F32)
        nc.sync.dma_start(c_consts[:], consts[:])
        neg2alpha = c_consts[:, 0:1]
        twoalpha = c_consts[:, 1:2]

        c_identf = const_p.tile([128, 128], F32)
        nc.sync.dma_start(c_identf[:], ident_in[:])
        c_ident = const_p.tile([128, 128], BF16)
        nc.vector.tensor_copy(c_ident[:], c_identf[:])

        c_ones1 = const_p.tile([1, 128], BF16)
        nc.vector.memset(c_ones1[:], 1.0)
        c_ones2 = const_p.tile([2, 128], BF16)
        nc.vector.memset(c_ones2[:], 1.0)
        c_ones128 = const_p.tile([128, 128], F32)
        nc.vector.memset(c_ones128[:], 1.0)

        for g in range(NGRP):
            d_g = gp.tile([128, 5 * GRP], F32, tag="d_g")
            rowsum_g = gp.tile([128, 5 * GRP], F32, tag="rowsum_g")
            xTs = []
            rs_tots = []

            # ---- phase 1: load, cast, d, transpose ----
            for bp in range(GRP):
                b = g * GRP + bp
                xf = sb.tile([128, NCHUNK, M], F32, tag="xf")
                nc.sync.dma_start(
                    xf[:], x[b].rearrange("(r p) m -> p r m", p=128)
                )
                xb = sb.tile([128, NCHUNK, M], BF16, tag="xb")
                nc.vector.tensor_copy(xb[:], xf[:])

                sqscr = sb.tile([128, M], F32, tag="sqscr")
                for r in range(NCHUNK):
                    nc.scalar.activation(
                        sqscr[:], xb[:, r, :], AF.Square,
                        accum_out=d_g[:, 5 * bp + r : 5 * bp + r + 1],
                    )

                xps = ps_xp.tile([M, DIM], BF16, tag="xp")
                for r in range(NCHUNK):
                    nc.tensor.transpose(
                        xps[:, r * 128 : (r + 1) * 128], xb[:, r, :], c_ident[:]
                    )
                xT = xtp.tile([M, DIM], BF16, tag="xT")
                nc.vector.tensor_copy(xT[:], xps[:])
                xTs.append(xT)

            # ---- phase 2: hneg, bias, t5 ----
            hneg_g = gp.tile([128, 5 * GRP], BF16, tag="hneg_g")
            nc.vector.tensor_scalar(
                out=hneg_g[:], in0=d_g[:], scalar1=-0.5, scalar2=None, op0=OP.mult
            )
            tmpb_g = gp.tile([128, 5 * GRP], F32, tag="tmpb_g")
            nc.vector.tensor_add(tmpb_g[:], d_g[:], hneg_g[:])
            bias_g = gp.tile([128, 5 * GRP], F32, tag="bias_g")
            nc.vector.tensor_scalar(
                out=bias_g[:], in0=tmpb_g[:], scalar1=twoalpha, scalar2=1e-5,
                op0=OP.mult, op1=OP.add,
            )
            xps2 = ps_xp.tile([5 * GRP, 128], BF16, tag="xp")
            nc.tensor.transpose(xps2[:], hneg_g[:], c_ident[:])
            t5_g = gp.tile([5 * GRP, 128], BF16, tag="t5_g")
            nc.vector.tensor_copy(t5_g[:], xps2[:])
            hrows = []
            for bp in range(GRP):
                hrow = psamp.tile([1, DIM], BF16, tag="hrow")
                nc.sync.dma_start(hrow[:], t5_g[5 * bp : 5 * bp + 5, :])
                hrows.append(hrow)

            # ---- phase 3: Gram + aug matmuls, sqrt, rowsums ----
            zs = []
            for bp in range(GRP):
                b = g * GRP + bp
                xT = xTs[bp]
                z = zpool.tile([128, NCHUNK, DIM], F32, tag="z")
                for r in range(NCHUNK):
                    lhsT = xT[:, r * 128 : (r + 1) * 128]
                    ps = ps_gram.tile([128, DIM], F32, tag="gram")
                    nc.tensor.matmul(
                        ps[:, 0:512], lhsT, xT[:, 0:512],
                        start=True, stop=False, skip_group_check=True,
                    )
                    nc.tensor.matmul(
                        ps[:, 512:640], lhsT, xT[:, 512:640],
                        start=True, stop=False, skip_group_check=True,
                    )
                    nc.tensor.matmul(
                        ps[:, 0:512], c_ones1[:], hrows[bp][:, 0:512],
                        start=False, stop=True, skip_group_check=True,
                    )
                    nc.tensor.matmul(
                        ps[:, 512:640], c_ones1[:], hrows[bp][:, 512:640],
                        start=False, stop=True, skip_group_check=True,
                    )
                    nc.scalar.activation(
                        z[:, r, :], ps[:], AF.Sqrt,
                        bias=bias_g[:, 5 * bp + r : 5 * bp + r + 1],
                        scale=neg2alpha,
                        accum_out=rowsum_g[:, 5 * bp + r : 5 * bp + r + 1],
                    )
                zs.append(z)
                rs_tot = psamp.tile([128, 1], F32, tag="rs_tot")
                nc.vector.tensor_reduce(
                    rs_tot[:], rowsum_g[:, 5 * bp : 5 * bp + 5],
                    axis=mybir.AxisListType.X, op=OP.add,
                )
                rs_tots.append(rs_tot)

            # ---- phase 4: row means, bf16 hi/lo (hi = cols 0:20, lo = 20:40), t10 ----
            rm_g = gp.tile([128, 5 * GRP], F32, tag="rm_g")
            nc.vector.tensor_scalar(
                out=rm_g[:], in0=rowsum_g[:], scalar1=1.0 / DIM, scalar2=None,
                op0=OP.mult,
            )
            rmstack_g = gp.tile([128, 2 * 5 * GRP], BF16, tag="rmstack_g")
            nc.vector.tensor_copy(rmstack_g[:, 0 : 5 * GRP], rm_g[:])
            rml_g = gp.tile([128, 5 * GRP], F32, tag="rml_g")
            nc.vector.tensor_sub(rml_g[:], rm_g[:], rmstack_g[:, 0 : 5 * GRP])
            nc.vector.tensor_copy(rmstack_g[:, 5 * GRP : 10 * GRP], rml_g[:])
            xps3 = ps_xp.tile([2 * 5 * GRP, 128], BF16, tag="xp")
            nc.tensor.transpose(xps3[:], rmstack_g[:], c_ident[:])
            t10_g = gp.tile([2 * 5 * GRP, 128], BF16, tag="t10_g")
            nc.vector.tensor_copy(t10_g[:], xps3[:])
            mrows = []
            for bp in range(GRP):
                mrow_hi = psamp.tile([1, DIM], BF16, tag="mrow_hi")
                nc.sync.dma_start(mrow_hi[:], t10_g[5 * bp : 5 * bp + 5, :])
                mrow_lo = psamp.tile([1, DIM], BF16, tag="mrow_lo")
                nc.sync.dma_start(
                    mrow_lo[:], t10_g[5 * GRP + 5 * bp : 5 * GRP + 5 * bp + 5, :]
                )
                mrows.append((mrow_hi, mrow_lo))

            # ---- phase 5: M matmuls, tm, final centering, DMA out ----
            for bp in range(GRP):
                b = g * GRP + bp
                z = zs[bp]
                mps = ps_m.tile([128, DIM], F32, tag="mps")
                nc.tensor.matmul(
                    mps[:, 0:1], c_ones128[:], rs_tots[bp][:],
                    start=True, stop=True, skip_group_check=True,
                )
                tm_b = psamp.tile([128, 1], F32, tag="tm_b")
                nc.scalar.mul(tm_b[:], mps[:, 0:1], 1.0 / (DIM * DIM))
                s0_b = psamp.tile([128, 5], F32, tag="s0_b")
                nc.vector.tensor_scalar(
                    out=s0_b[:], in0=rm_g[:, 5 * bp : 5 * bp + 5],
                    scalar1=tm_b[:], scalar2=None, op0=OP.subtract,
                )
                mrow_hi, mrow_lo = mrows[bp]
                nc.tensor.matmul(
                    mps[:, 0:512], c_ones1[:], mrow_hi[:, 0:512],
                    start=True, stop=False, skip_group_check=True,
                )
                nc.tensor.matmul(
                    mps[:, 512:640], c_ones1[:], mrow_hi[:, 512:640],
                    start=True, stop=False, skip_group_check=True,
                )
                nc.tensor.matmul(
                    mps[:, 0:512], c_ones1[:], mrow_lo[:, 0:512],
                    start=False, stop=True, skip_group_check=True,
                )
                nc.tensor.matmul(
                    mps[:, 512:640], c_ones1[:], mrow_lo[:, 512:640],
                    start=False, stop=True, skip_group_check=True,
                )
                for r in range(NCHUNK):
                    outt = opool.tile([128, DIM], F32, tag="outt")
                    nc.vector.scalar_tensor_tensor(
                        outt[:], z[:, r, :], s0_b[:, r : r + 1], mps[:],
                        op0=OP.subtract, op1=OP.subtract,
                    )
                    nc.sync.dma_start(
                        out[b, r * 128 * DIM : (r + 1) * 128 * DIM].rearrange(
                            "(p e) -> p e", p=128
                        ),
                        outt[:],
                    )

    nc.compile()
    return nc


def _get_nc():
    global _cached_nc
    if _cached_nc is None:
        _cached_nc = build()
    return _cached_nc


def make_in_maps(x: np.ndarray, t: np.ndarray):
    alpha = float(np.exp(t.astype(np.float64))[0, 0])
    consts = np.zeros((128, 2), dtype=np.float32)
    consts[:, 0] = -2.0 * alpha
    consts[:, 1] = 2.0 * alpha
    ident = np.eye(128, dtype=np.float32)
    xs = x.reshape(N_CORES, B_CORE, DIM, M)
    return [
        {"x": np.ascontiguousarray(xs[c]), "consts": consts, "ident": ident}
        for c in range(N_CORES)
    ]


def kernel(x: np.ndarray, t: np.ndarray) -> np.ndarray:
    x = np.asarray(x, dtype=np.float32)
    t = np.asarray(t, dtype=np.float32)
    nc = _get_nc()
    res = run_bass_kernel_spmd(nc, make_in_maps(x, t), core_ids=list(range(N_CORES)))
    return np.concatenate([r["out"] for r in res.results], axis=0)


# revision 35
# speedup vs baseline: 53832.4795x; 53832.4795x over previous
"""Trainium2 Bass kernel for per-sample Brownian-distance-covariance (BDC) pooling.

Problem: x [128, 640, 100] f32, t [1,1] f32 (log temperature).
  per sample: G = x @ x^T; dcov = d_i + d_j - 2G; dcov = max(dcov, 1e-4);
  z = sqrt(exp(t)*dcov + 1e-5); out = z - rowmean - colmean + totmean.
Output: [128, 409600] f32.

Strategy (8 NeuronCores, pure data parallel, 16 samples/core):
  - x cast to bf16; Gram via TensorE (contraction over the 100-dim on
    partitions, operands transposed on-chip via TensorE identity-matmul).
  - The per-sample distance matrix is symmetric -> column means == row means,
    so no column reduction is needed: row sums come free from the ScalarE
    sqrt pass (activation accum_out).
  - d_j enters the Gram PSUM via a rank-1 bf16 matmul of the transposed
    d-vector; the per-partition activation bias compensates the bf16
    rounding of that row exactly on the diagonal, so the clamp is never
    active off-diagonal and the diagonal lands at sqrt(1e-5) (vs reference
    sqrt(exp(t)*1e-4 + 1e-5) -- a 7.8e-5 absolute difference on 640 of
    409600 elements, far below tolerance).
  - Double centering fused into one VectorE scalar_tensor_tensor pass:
    out = (z - (rowmean[p] - totmean)) - M[p, e], with M = colmean broadcast
    built by tiny rank-2 matmuls from a bf16 hi/lo split of the row means.
"""
import numpy as np
from contextlib import ExitStack

import concourse.bass as bass
import concourse.bacc as bacc
import concourse.tile as tile
from concourse import mybir
from concourse.bass_utils import run_bass_kernel_spmd

F32 = mybir.dt.float32
BF16 = mybir.dt.bfloat16
AF = mybir.ActivationFunctionType
OP = mybir.AluOpType

N_CORES = 8
B_TOTAL = 128
B_CORE = B_TOTAL // N_CORES  # 16
DIM = 640
M = 100
NCHUNK = DIM // 128  # 5
GROUPS = [2] * 8   # samples per phase-group (sums to B_CORE)
assert sum(GROUPS) == B_CORE
GRP_MAX = max(GROUPS)

_cached_nc = None


def build():
    nc = bacc.Bacc("TRN2", target_bir_lowering=False)
    x = nc.dram_tensor("x", [B_CORE, DIM, M], F32, kind="ExternalInput")
    consts = nc.dram_tensor("consts", [128, 2], F32, kind="ExternalInput")
    ident_in = nc.dram_tensor("ident", [128, 128], F32, kind="ExternalInput")
    out = nc.dram_tensor("out", [B_CORE, DIM * DIM], F32, kind="ExternalOutput")

    with tile.TileContext(nc) as tc, ExitStack() as ctx:
        const_p = ctx.enter_context(tc.tile_pool(name="const", bufs=1))
        sb = ctx.enter_context(tc.tile_pool(name="sb", bufs=3))
        zpool = ctx.enter_context(tc.tile_pool(name="zp", bufs=6))
        xtp = ctx.enter_context(tc.tile_pool(name="xtp", bufs=6))
        psamp = ctx.enter_context(tc.tile_pool(name="psamp", bufs=8))
        opool = ctx.enter_context(tc.tile_pool(name="op", bufs=4))
        gp = ctx.enter_context(tc.tile_pool(name="gp", bufs=2))
        ps_gram = ctx.enter_context(tc.tile_pool(name="psg", bufs=2, space="PSUM"))
        ps_m = ctx.enter_context(tc.tile_pool(name="psm", bufs=1, space="PSUM"))
        ps_xp = ctx.enter_context(tc.tile_pool(name="psx", bufs=1, space="PSUM"))

        # ---- constants ----
        c_consts = const_p.tile([128, 2], F32)
        nc.sync.dma_start(c_consts[:], consts[:])
        neg2alpha = c_consts[:, 0:1]
        twoalpha = c_consts[:, 1:2]

        c_identf = const_p.tile([128, 128], F32)
        nc.sync.dma_start(c_identf[:], ident_in[:])
        c_ident = const_p.tile([128, 128], BF16)
        nc.vector.tensor_copy(c_ident[:], c_identf[:])

        c_ones1 = const_p.tile([1, 128], BF16)
        nc.vector.memset(c_ones1[:], 1.0)
        c_ones2 = const_p.tile([2, 128], BF16)
        nc.vector.memset(c_ones2[:], 1.0)
        c_ones128 = const_p.tile([128, 128], F32)
        nc.vector.memset(c_ones128[:], 1.0)

        def emit_tail_prep(b0, gsz, rowsum_g):
            NQ = 5 * gsz
            rm_g = gp.tile([128, NQ], F32, tag="rm_g")
            nc.vector.tensor_scalar(
                out=rm_g[:], in0=rowsum_g[:], scalar1=1.0 / DIM, scalar2=None,
                op0=OP.mult,
            )
            rmstack_g = gp.tile([128, 2 * NQ], BF16, tag="rmstack_g")
            nc.vector.tensor_copy(rmstack_g[:, 0:NQ], rm_g[:])
            rml_g = gp.tile([128, NQ], F32, tag="rml_g")
            nc.vector.tensor_sub(rml_g[:], rm_g[:], rmstack_g[:, 0:NQ])
            nc.vector.tensor_copy(rmstack_g[:, NQ : 2 * NQ], rml_g[:])
            xps3 = ps_xp.tile([2 * NQ, 128], BF16, tag="xp2")
            nc.tensor.transpose(xps3[:], rmstack_g[:], c_ident[:])
            t10_g = gp.tile([2 * NQ, 128], BF16, tag="t10_g")
            nc.vector.tensor_copy(t10_g[:], xps3[:])
            mrows = []
            for bp in range(gsz):
                mrow = psamp.tile([2, DIM], BF16, tag="mrow")
                nc.sync.dma_start(mrow[0:1, :], t10_g[5 * bp : 5 * bp + 5, :])
                nc.sync.dma_start(
                    mrow[1:2, :], t10_g[NQ + 5 * bp : NQ + 5 * bp + 5, :]
                )
                mrows.append(mrow)
            return rm_g, mrows

        def emit_tail_sample(b0, gsz, bp, zs, rs_tots, rm_g, mrows):
            b = b0 + bp
            z = zs[bp]
            mps = ps_m.tile([128, DIM], F32, tag="mps")
            nc.tensor.matmul(
                mps[:, 0:1], c_ones128[:], rs_tots[bp][:],
                start=True, stop=True, skip_group_check=True,
            )
            tm_b = psamp.tile([128, 1], F32, tag="tm_b")
            nc.scalar.mul(tm_b[:], mps[:, 0:1], 1.0 / (DIM * DIM))
            s0_b = psamp.tile([128, 5], F32, tag="s0_b")
            nc.vector.tensor_scalar(
                out=s0_b[:], in0=rm_g[:, 5 * bp : 5 * bp + 5],
                scalar1=tm_b[:], scalar2=None, op0=OP.subtract,
            )
            mrow = mrows[bp]
            nc.tensor.matmul(
                mps[:, 0:512], c_ones2[:], mrow[:, 0:512],
                start=True, stop=True, skip_group_check=True,
            )
            nc.tensor.matmul(
                mps[:, 512:640], c_ones2[:], mrow[:, 512:640],
                start=True, stop=True, skip_group_check=True,
            )
            m_sb = psamp.tile([128, DIM], F32, tag="m_sb")
            nc.vector.tensor_copy(m_sb[:], mps[:])
            for r in range(NCHUNK):
                outt = opool.tile([128, DIM], F32, tag="outt")
                nc.vector.scalar_tensor_tensor(
                    outt[:], z[:, r, :], s0_b[:, r : r + 1], m_sb[:],
                    op0=OP.subtract, op1=OP.subtract,
                )
                nc.sync.dma_start(
                    out[b, r * 128 * DIM : (r + 1) * 128 * DIM].rearrange(
                        "(p e) -> p e", p=128
                    ),
                    outt[:],
                )

        pending = None
        b0 = 0
        for gsz in GROUPS:
            NQ = 5 * gsz
            d_g = gp.tile([128, NQ], F32, tag="d_g")
            rowsum_g = gp.tile([128, NQ], F32, tag="rowsum_g")
            xTs = []
            rs_tots = []

            # ---- phase 1: load+cast DMA, d, transpose ----
            for bp in range(gsz):
                b = b0 + bp
                xb = sb.tile([128, NCHUNK, M], BF16, tag="xb")
                nc.gpsimd.dma_start(
                    xb[:], x[b].rearrange("(r p) m -> p r m", p=128)
                )
                sqscr = sb.tile([128, M], F32, tag="sqscr")
                for r in range(NCHUNK):
                    nc.scalar.activation(
                        sqscr[:], xb[:, r, :], AF.Square,
                        accum_out=d_g[:, 5 * bp + r : 5 * bp + r + 1],
                    )
                xps = ps_xp.tile([M, DIM], BF16, tag="xp")
                for r in range(NCHUNK):
                    nc.tensor.transpose(
                        xps[:, r * 128 : (r + 1) * 128], xb[:, r, :], c_ident[:]
                    )
                xT = xtp.tile([M, DIM], BF16, tag="xT")
                nc.vector.tensor_copy(xT[:], xps[:])
                xTs.append(xT)

            # ---- phase 2: hneg hi/lo, bias, hrows ----
            hstack_g = gp.tile([128, 2 * NQ], BF16, tag="hstack_g")
            nc.vector.tensor_scalar(
                out=hstack_g[:, 0:NQ], in0=d_g[:], scalar1=-0.5, scalar2=None,
                op0=OP.mult,
            )
            hres_g = gp.tile([128, NQ], F32, tag="hres_g")
            nc.vector.tensor_scalar(
                out=hres_g[:], in0=d_g[:], scalar1=-0.5, scalar2=None, op0=OP.mult
            )
            nc.vector.tensor_sub(hres_g[:], hres_g[:], hstack_g[:, 0:NQ])
            nc.vector.tensor_copy(hstack_g[:, NQ : 2 * NQ], hres_g[:])
            tmpb_g = gp.tile([128, NQ], F32, tag="tmpb_g")
            nc.vector.tensor_add(tmpb_g[:], d_g[:], hstack_g[:, 0:NQ])
            nc.vector.tensor_add(tmpb_g[:], tmpb_g[:], hstack_g[:, NQ : 2 * NQ])
            bias_g = gp.tile([128, NQ], F32, tag="bias_g")
            nc.vector.tensor_scalar(
                out=bias_g[:], in0=tmpb_g[:], scalar1=twoalpha, scalar2=1e-5,
                op0=OP.mult, op1=OP.add,
            )
            xps2 = ps_xp.tile([2 * NQ, 128], BF16, tag="xp2")
            nc.tensor.transpose(xps2[:], hstack_g[:], c_ident[:])
            t5_g = gp.tile([2 * NQ, 128], BF16, tag="t5_g")
            nc.vector.tensor_copy(t5_g[:], xps2[:])
            hrows = []
            for bp in range(gsz):
                hrow = psamp.tile([2, DIM], BF16, tag="hrow")
                nc.sync.dma_start(hrow[0:1, :], t5_g[5 * bp : 5 * bp + 5, :])
                nc.sync.dma_start(hrow[1:2, :], t5_g[NQ + 5 * bp : NQ + 5 * bp + 5, :])
                hrows.append(hrow)

            # ---- tail of previous group overlaps this group's phase 3 ----
            tailq = []
            if pending is not None:
                pb0, pgsz, pzs, prs, prowsum = pending
                prm_g, pmrows = emit_tail_prep(pb0, pgsz, prowsum)
                tailq = [
                    (pb0, pgsz, k, pzs, prs, prm_g, pmrows) for k in range(pgsz)
                ]

            # ---- phase 3: Gram + aug matmuls, sqrt, rowsums ----
            zs = []
            for bp in range(gsz):
                xT = xTs[bp]
                z = zpool.tile([128, NCHUNK, DIM], F32, tag="z")
                for r in range(NCHUNK):
                    lhsT = xT[:, r * 128 : (r + 1) * 128]
                    ps = ps_gram.tile([128, DIM], F32, tag="gram")
                    nc.tensor.matmul(
                        ps[:, 0:512], lhsT, xT[:, 0:512],
                        start=True, stop=False, skip_group_check=True,
                    )
                    nc.tensor.matmul(
                        ps[:, 512:640], lhsT, xT[:, 512:640],
                        start=True, stop=False, skip_group_check=True,
                    )
                    nc.tensor.matmul(
                        ps[:, 0:512], c_ones2[:], hrows[bp][:, 0:512],
                        start=False, stop=True, skip_group_check=True,
                    )
                    nc.tensor.matmul(
                        ps[:, 512:640], c_ones2[:], hrows[bp][:, 512:640],
                        start=False, stop=True, skip_group_check=True,
                    )
                    nc.scalar.activation(
                        z[:, r, :], ps[:], AF.Sqrt,
                        bias=bias_g[:, 5 * bp + r : 5 * bp + r + 1],
                        scale=neg2alpha,
                        accum_out=rowsum_g[:, 5 * bp + r : 5 * bp + r + 1],
                    )
                zs.append(z)
                rs_tot = psamp.tile([128, 1], F32, tag="rs_tot")
                nc.vector.tensor_reduce(
                    rs_tot[:], rowsum_g[:, 5 * bp : 5 * bp + 5],
                    axis=mybir.AxisListType.X, op=OP.add,
                )
                rs_tots.append(rs_tot)
                while tailq and len(tailq) * gsz > (gsz - 1 - bp) * len(zs) + len(tailq) * 0:
                    if len(tailq) <= gsz - 1 - bp:
                        break
                    emit_tail_sample(*tailq.pop(0))

            for item in tailq:
                emit_tail_sample(*item)
            if b0 == 0:
                frm_g, fmrows = emit_tail_prep(b0, gsz, rowsum_g)
                for k in range(gsz):
                    emit_tail_sample(b0, gsz, k, zs, rs_tots, frm_g, fmrows)
                pending = None
            else:
                pending = (b0, gsz, zs, rs_tots, rowsum_g)
            b0 += gsz

        pb0, pgsz, pzs, prs, prowsum = pending
        prm_g, pmrows = emit_tail_prep(pb0, pgsz, prowsum)
        for k in range(pgsz):
            emit_tail_sample(pb0, pgsz, k, pzs, prs, prm_g, pmrows)

    nc.compile()
    return nc


def _get_nc():
    global _cached_nc
    if _cached_nc is None:
        _cached_nc = build()
    return _cached_nc


def make_in_maps(x: np.ndarray, t: np.ndarray):
    alpha = float(np.exp(t.astype(np.float64))[0, 0])
    consts = np.zeros((128, 2), dtype=np.float32)
    consts[:, 0] = -2.0 * alpha
    consts[:, 1] = 2.0 * alpha
    ident = np.eye(128, dtype=np.float32)
    xs = x.reshape(N_CORES, B_CORE, DIM, M)
    return [
        {"x": np.ascontiguousarray(xs[c]), "consts": consts, "ident": ident}
        for c in range(N_CORES)
    ]


def kernel(x: np.ndarray, t: np.ndarray) -> np.ndarray:
    x = np.asarray(x, dtype=np.float32)
    t = np.asarray(t, dtype=np.float32)
    nc = _get_nc()
    res = run_bass_kernel_spmd(nc, make_in_maps(x, t), core_ids=list(range(N_CORES)))
    return np.concatenate([r["out"] for r in res.results], axis=0)
